# revision 12
# baseline (speedup 1.0000x reference)
"""Trainium2 Bass kernel for nn_CINLayer: out[b,d,o] = sum_{n,m} x[b,d,n]*y[b,d,m]*W[o,n*M+m].

Strategy (8-core data parallel over batch), v3:
  Per sample s, out[s,o] = sum_k Z[k,s] * Wl[k,o] with Z[k,s] = x[s,n(k)]*y[s,m(k)].
  The 1600 (n,m) products are covered by 13 chunks of 128 rows, each chunk a
  product set P x Q so its Z is ONE elementwise multiply of two broadcast
  layouts:
    part A (c=2a+b<10):  row r: (n,m) = (8a + r//16, 16b + r%16)
        z_c = XA_a * YA_b,  XA_a[r]=xT[8a+r//16], YA_b[r]=yT[16b+r%16]
        XA_a / YA_b host-staged, DMA'd from HBM.
    part B (c=10+cb):    row r=32j+8a'+m'': (n,m) = (16cb+4a'+j, 32+m'')
        [n>=40 rows zeroed in W]
        z_c = XB_cb * YB,  YB[r]=yT[32+r%8] host-staged; XB_cb built on-device
        by one DVE stream_shuffle (mask[r']=4cb+r'//8) of a host-staged
        interleaved base xil[32j+i]=xT[4i+j], run on an int32 bitcast view to
        halve the 1x-mode shuffle cost.
  Layouts for each sample group are staged contiguously in DRAM (~KB-scale
  per-partition rows) so input DMA runs near peak BW. Group widths are small
  at the start/end of the kernel to shorten pipeline fill/drain.
  PE runs z-stationary matmuls: lhsT = z chunk slice [128k, 128 samples]
  (128-wide stationary -> fast weight load), moving = W chunk [128k, 200o],
  accumulating psum[128 samples, 200 o] over the 13 chunks (no o=200->128+72
  two-pass padding). One mul per group runs on GPSIMD to offload DVE.
  Output: psum -> f16 SBUF tiles packing 4 sample-blocks [128, 800] -> HBM.
"""

import numpy as np

BS, DIM, N, M, O = 2048, 32, 40, 40, 200
NCORES = 8
S_PER_CORE = BS * DIM // NCORES  # 8192
NCHUNKS = 13
NLAY = 11                  # staged: XA0,YA0,XA1..XA4,YA1,YB,XB0..XB2
LAY_YB = 7
LAY_XIL = 8                # in 9-layout (shuffle) mode, slot 8 holds XIL
GROUPS = [512, 512] + [1024] * 7
# early groups use the 9-layout shuffle mode (fewer input bytes while the
# pipeline fills and HBM bandwidth is the binding constraint)
GNLAY = [9 if g < 4 else NLAY for g in range(len(GROUPS))]
assert sum(GROUPS) == S_PER_CORE
GOFF = np.concatenate([[0], np.cumsum([nl * w for nl, w in zip(GNLAY, GROUPS)])])
BLK = 128
WMAX = 1024
F16 = np.float16

# chunk c -> (x layout index, y layout index)
_XA_IDX = [0, 2, 3, 4, 5]
CHUNK_LAYS = [(_XA_IDX[c // 2], 1 if c % 2 == 0 else 6) for c in range(10)] + [
    (8 + cb, LAY_YB) for cb in range(3)
]
# mul issue order: even-c (sub-DMA 1), odd-c, then part B
MORDER = [0, 2, 4, 6, 8, 1, 3, 5, 7, 9, 10, 11, 12]
GPSIMD_MULS = frozenset()


def _shuffle_mask(cb: int):
    return [4 * cb + (rp // 8) for rp in range(32)]


def _chunk_row_to_nm(c: int, r: int):
    """Chunk c (0..12), row r (0..127) -> (n, m) or None (zero pad)."""
    if c < 10:
        return 8 * (c // 2) + r // 16, 16 * (c % 2) + r % 16
    cb = c - 10
    j, rp = divmod(r, 32)
    ap, mpp = divmod(rp, 8)
    n = 16 * cb + 4 * ap + j
    if n >= N:
        return None
    return n, 32 + mpp


def _stage_w(W: np.ndarray) -> np.ndarray:
    """W [O, N*M] f32 -> wl [128, NCHUNKS, O] f16 (z-stationary moving operand)."""
    Wr = W.reshape(O, N, M)
    wl = np.zeros((128, NCHUNKS, O), dtype=F16)
    for c in range(NCHUNKS):
        for r in range(128):
            nm = _chunk_row_to_nm(c, r)
            if nm is not None:
                wl[r, c, :] = Wr[:, nm[0], nm[1]].astype(F16)
    return wl


def _lay_row_maps():
    """11 staged layouts: list of (which, idx[128]) with idx=-1 meaning zero row."""
    r = np.arange(128)
    xa = [("x", 8 * a + r // 16) for a in range(5)]
    ya = [("y", 16 * b + r % 16) for b in range(2)]
    yb = ("y", 32 + r % 8)
    j, rp = r // 32, r % 32
    ap = rp // 8
    xb = []
    for cb in range(3):
        idx = 16 * cb + 4 * ap + j
        xb.append(("x", np.where(idx < N, idx, -1)))
    xil = ("x", np.where(rp < 10, 4 * rp + j, -1))
    return [xa[0], ya[0], xa[1], xa[2], xa[3], xa[4], ya[1], yb] + xb + [xil]


_LAY_MAPS = _lay_row_maps()


def _stage_core_inputs(x_flat: np.ndarray, y_flat: np.ndarray) -> np.ndarray:
    """x_flat, y_flat [S_PER_CORE, 40] f32 -> xg [128, sum(NLAY*Wg)] f16."""
    xT = np.ascontiguousarray(x_flat.T).astype(F16)  # [40, S]
    yT = np.ascontiguousarray(y_flat.T).astype(F16)  # [40, S]
    src = {"x": xT, "y": yT}
    lays = np.empty((len(_LAY_MAPS), 128, S_PER_CORE), dtype=F16)
    for li, (which, idx) in enumerate(_LAY_MAPS):
        t = src[which]
        lays[li] = np.where((idx >= 0)[:, None], t[np.clip(idx, 0, N - 1)], F16(0))
    xg = np.empty((128, GOFF[-1]), dtype=F16)
    s0 = 0
    for g, w in enumerate(GROUPS):
        sel = list(range(8)) + ([11] if GNLAY[g] == 9 else [8, 9, 10])
        blk = lays[sel][:, :, s0 : s0 + w].transpose(1, 0, 2)
        xg[:, GOFF[g] : GOFF[g + 1]] = blk.reshape(128, GNLAY[g] * w)
        s0 += w
    return xg


def unpack_out(outt: np.ndarray) -> np.ndarray:
    """outt [16, 128, 4*O] f16 -> [S_PER_CORE, O] f32."""
    o4 = outt.reshape(-1, 128, 4, O).transpose(0, 2, 1, 3)
    return o4.reshape(S_PER_CORE, O).astype(np.float32)


def build_nc(debug: bool = False):
    """Build the per-core Bass/Tile module."""
    import concourse.bass as bass
    import concourse.tile as tile
    from concourse import bacc, mybir

    f16 = mybir.dt.float16
    f32 = mybir.dt.float32
    i32 = mybir.dt.int32

    nc = bacc.Bacc("TRN2", target_bir_lowering=False, debug=debug)

    xg_d = nc.dram_tensor("xg", [128, int(GOFF[-1])], f16, kind="ExternalInput")
    wl_d = nc.dram_tensor("wl", [128, NCHUNKS, O], f16, kind="ExternalInput")
    out_d = nc.dram_tensor("outt", [16, 128, 4 * O], f16, kind="ExternalOutput")

    with tile.TileContext(nc) as tc:
        with (
            tc.tile_pool(name="wpool", bufs=1) as wpool,
            tc.tile_pool(name="lay", bufs=3) as laypool,
            tc.tile_pool(name="xep", bufs=6) as xep,
            tc.tile_pool(name="zp", bufs=30) as zp,
            tc.tile_pool(name="outp", bufs=4) as outp,
            tc.tile_pool(name="ps", bufs=8, space=bass.MemorySpace.PSUM) as psp,
        ):
            wl_sb = wpool.tile([128, NCHUNKS, O], f16)
            nc.sync.dma_start(wl_sb[:], wl_d[:])

            ot_idx = 0
            blk_seq = 0
            prev_lay = None
            for g, wg in enumerate(GROUPS):
                nlay = GNLAY[g]
                lay = laypool.tile([128, NLAY * WMAX], f16)

                def lsl(li, w=wg):
                    return lay[:, li * w : (li + 1) * w]

                o0 = int(GOFF[g])
                if g == 0:
                    # minimal first sub-DMA (XA0+YA0) so chunk 0's mul and the
                    # first PE chain start as early as possible
                    nc.sync.dma_start(lay[:, 0 : 2 * wg], xg_d[:, o0 : o0 + 2 * wg])
                    nc.sync.dma_start(
                        lay[:, 2 * wg : 6 * wg], xg_d[:, o0 + 2 * wg : o0 + 6 * wg]
                    )
                    nc.sync.dma_start(
                        lay[:, 6 * wg : nlay * wg], xg_d[:, o0 + 6 * wg : o0 + nlay * wg]
                    )
                elif wg >= 1024:
                    # split so early (even-c) chunks can start sooner
                    nc.sync.dma_start(lay[:, 0 : 6 * wg], xg_d[:, o0 : o0 + 6 * wg])
                    nc.sync.dma_start(
                        lay[:, 6 * wg : nlay * wg], xg_d[:, o0 + 6 * wg : o0 + nlay * wg]
                    )
                else:
                    nc.sync.dma_start(lay[:, 0 : nlay * wg], xg_d[:, o0 : o0 + nlay * wg])

                xe = {}
                z = {}

                def mul(c):
                    zc = zp.tile([128, WMAX], f16, tag="z")
                    xi, yi = CHUNK_LAYS[c]
                    if c >= 10 and nlay == 9:
                        xin = xe[c - 10][:, 0:wg]
                    else:
                        xin = lsl(xi)
                    nc.vector.tensor_mul(zc[:, 0:wg], lsl(yi), xin)
                    z[c] = zc

                for c in MORDER[:5]:
                    mul(c)
                if nlay == 9:
                    for cb in range(3):
                        t = xep.tile([128, WMAX], f16, tag="xe")
                        nc.vector.stream_shuffle(
                            t[:, 0:wg].bitcast(i32),
                            lsl(LAY_XIL).bitcast(i32),
                            _shuffle_mask(cb),
                        )
                        xe[cb] = t
                for c in MORDER[5:]:
                    mul(c)

                for k4 in range(wg // 512):
                    ot = outp.tile([128, 4 * O], f16)
                    for kk in range(4):
                        blk = 4 * k4 + kk
                        ps = psp.tile([128, 512], f32)
                        r0 = blk_seq % NCHUNKS
                        blk_seq += 1
                        rot = MORDER[r0:] + MORDER[:r0]
                        sl = slice(blk * BLK, (blk + 1) * BLK)
                        for i, c in enumerate(rot):
                            nc.tensor.matmul(
                                ps[:, 0:O], z[c][:, sl], wl_sb[:, c, :],
                                start=(i == 0), stop=(i == NCHUNKS - 1),
                            )
                        nc.scalar.copy(ot[:, kk * O : (kk + 1) * O], ps[:, 0:O])
                    nc.scalar.dma_start(out_d[ot_idx], ot[:])
                    ot_idx += 1

    nc.compile()
    return nc


def prepare_in_maps(x: np.ndarray, y: np.ndarray, W: np.ndarray):
    wl = _stage_w(W)
    x_cores = x.reshape(NCORES, S_PER_CORE, N)
    y_cores = y.reshape(NCORES, S_PER_CORE, M)
    in_maps = []
    for i in range(NCORES):
        xg = _stage_core_inputs(x_cores[i], y_cores[i])
        in_maps.append({"xg": xg, "wl": wl})
    return in_maps


def kernel(x: np.ndarray, y: np.ndarray, W: np.ndarray) -> np.ndarray:
    from concourse.bass_utils import run_bass_kernel_spmd

    assert x.shape == (BS, DIM, N) and y.shape == (BS, DIM, M)
    assert W.shape == (O, N * M)

    in_maps = prepare_in_maps(x, y, W)
    nc = build_nc()
    res = run_bass_kernel_spmd(nc, in_maps, core_ids=list(range(NCORES)))

    outs = [unpack_out(res.results[i]["outt"]) for i in range(NCORES)]
    return np.concatenate(outs, axis=0).reshape(BS, DIM, O)


if __name__ == "__main__":
    xs = np.random.randn(BS, DIM, N).astype(np.float32)
    ys = np.random.randn(BS, DIM, M).astype(np.float32)
    Ws = (np.random.randn(O, N * M) * (1.0 / np.sqrt(N * M))).astype(np.float32)
    out = kernel(xs, ys, Ws)
    print(out.shape, out.dtype)


# revision 13
# speedup vs baseline: 1.0230x; 1.0230x over previous
"""Trainium2 Bass kernel for nn_CINLayer: out[b,d,o] = sum_{n,m} x[b,d,n]*y[b,d,m]*W[o,n*M+m].

Strategy (8-core data parallel over batch), v3:
  Per sample s, out[s,o] = sum_k Z[k,s] * Wl[k,o] with Z[k,s] = x[s,n(k)]*y[s,m(k)].
  The 1600 (n,m) products are covered by 13 chunks of 128 rows, each chunk a
  product set P x Q so its Z is ONE elementwise multiply of two broadcast
  layouts:
    part A (c=2a+b<10):  row r: (n,m) = (8a + r//16, 16b + r%16)
        z_c = XA_a * YA_b,  XA_a[r]=xT[8a+r//16], YA_b[r]=yT[16b+r%16]
        XA_a / YA_b host-staged, DMA'd from HBM.
    part B (c=10+cb):    row r=32j+8a'+m'': (n,m) = (16cb+4a'+j, 32+m'')
        [n>=40 rows zeroed in W]
        z_c = XB_cb * YB,  YB[r]=yT[32+r%8] host-staged; XB_cb built on-device
        by one DVE stream_shuffle (mask[r']=4cb+r'//8) of a host-staged
        interleaved base xil[32j+i]=xT[4i+j], run on an int32 bitcast view to
        halve the 1x-mode shuffle cost.
  Layouts for each sample group are staged contiguously in DRAM (~KB-scale
  per-partition rows) so input DMA runs near peak BW. Group widths are small
  at the start/end of the kernel to shorten pipeline fill/drain.
  PE runs z-stationary matmuls: lhsT = z chunk slice [128k, 128 samples]
  (128-wide stationary -> fast weight load), moving = W chunk [128k, 200o],
  accumulating psum[128 samples, 200 o] over the 13 chunks (no o=200->128+72
  two-pass padding). One mul per group runs on GPSIMD to offload DVE.
  Output: psum -> f16 SBUF tiles packing 4 sample-blocks [128, 800] -> HBM.
"""

import numpy as np

BS, DIM, N, M, O = 2048, 32, 40, 40, 200
NCORES = 8
S_PER_CORE = BS * DIM // NCORES  # 8192
NCHUNKS = 13
NLAY = 11                  # staged: XA0,YA0,XA1..XA4,YA1,YB,XB0..XB2
LAY_YB = 7
LAY_XIL = 8                # in 9-layout (shuffle) mode, slot 8 holds XIL
GROUPS = [512, 512] + [1024] * 7
# early groups use the 9-layout shuffle mode (fewer input bytes while the
# pipeline fills and HBM bandwidth is the binding constraint)
GNLAY = [9 if g < 2 else NLAY for g in range(len(GROUPS))]
assert sum(GROUPS) == S_PER_CORE
GOFF = np.concatenate([[0], np.cumsum([nl * w for nl, w in zip(GNLAY, GROUPS)])])
BLK = 128
WMAX = 1024
F16 = np.float16

# chunk c -> (x layout index, y layout index)
_XA_IDX = [0, 2, 3, 4, 5]
CHUNK_LAYS = [(_XA_IDX[c // 2], 1 if c % 2 == 0 else 6) for c in range(10)] + [
    (8 + cb, LAY_YB) for cb in range(3)
]
# mul issue order: even-c (sub-DMA 1), odd-c, then part B
MORDER = [0, 2, 4, 6, 8, 1, 3, 5, 7, 9, 10, 11, 12]
GPSIMD_MULS = frozenset()


def _shuffle_mask(cb: int):
    return [4 * cb + (rp // 8) for rp in range(32)]


def _chunk_row_to_nm(c: int, r: int):
    """Chunk c (0..12), row r (0..127) -> (n, m) or None (zero pad)."""
    if c < 10:
        return 8 * (c // 2) + r // 16, 16 * (c % 2) + r % 16
    cb = c - 10
    j, rp = divmod(r, 32)
    ap, mpp = divmod(rp, 8)
    n = 16 * cb + 4 * ap + j
    if n >= N:
        return None
    return n, 32 + mpp


def _stage_w(W: np.ndarray) -> np.ndarray:
    """W [O, N*M] f32 -> wl [128, NCHUNKS, O] f16 (z-stationary moving operand)."""
    Wr = W.reshape(O, N, M)
    wl = np.zeros((128, NCHUNKS, O), dtype=F16)
    for c in range(NCHUNKS):
        for r in range(128):
            nm = _chunk_row_to_nm(c, r)
            if nm is not None:
                wl[r, c, :] = Wr[:, nm[0], nm[1]].astype(F16)
    return wl


def _lay_row_maps():
    """11 staged layouts: list of (which, idx[128]) with idx=-1 meaning zero row."""
    r = np.arange(128)
    xa = [("x", 8 * a + r // 16) for a in range(5)]
    ya = [("y", 16 * b + r % 16) for b in range(2)]
    yb = ("y", 32 + r % 8)
    j, rp = r // 32, r % 32
    ap = rp // 8
    xb = []
    for cb in range(3):
        idx = 16 * cb + 4 * ap + j
        xb.append(("x", np.where(idx < N, idx, -1)))
    xil = ("x", np.where(rp < 10, 4 * rp + j, -1))
    return [xa[0], ya[0], xa[1], xa[2], xa[3], xa[4], ya[1], yb] + xb + [xil]


_LAY_MAPS = _lay_row_maps()


def _stage_core_inputs(x_flat: np.ndarray, y_flat: np.ndarray) -> np.ndarray:
    """x_flat, y_flat [S_PER_CORE, 40] f32 -> xg [128, sum(NLAY*Wg)] f16."""
    xT = np.ascontiguousarray(x_flat.T).astype(F16)  # [40, S]
    yT = np.ascontiguousarray(y_flat.T).astype(F16)  # [40, S]
    src = {"x": xT, "y": yT}
    lays = np.empty((len(_LAY_MAPS), 128, S_PER_CORE), dtype=F16)
    for li, (which, idx) in enumerate(_LAY_MAPS):
        t = src[which]
        lays[li] = np.where((idx >= 0)[:, None], t[np.clip(idx, 0, N - 1)], F16(0))
    xg = np.empty((128, GOFF[-1]), dtype=F16)
    s0 = 0
    for g, w in enumerate(GROUPS):
        sel = list(range(8)) + ([11] if GNLAY[g] == 9 else [8, 9, 10])
        blk = lays[sel][:, :, s0 : s0 + w].transpose(1, 0, 2)
        xg[:, GOFF[g] : GOFF[g + 1]] = blk.reshape(128, GNLAY[g] * w)
        s0 += w
    return xg


def unpack_out(outt: np.ndarray) -> np.ndarray:
    """outt [16, 128, 4*O] f16 -> [S_PER_CORE, O] f32."""
    o4 = outt.reshape(-1, 128, 4, O).transpose(0, 2, 1, 3)
    return o4.reshape(S_PER_CORE, O).astype(np.float32)


def build_nc(debug: bool = False):
    """Build the per-core Bass/Tile module."""
    import concourse.bass as bass
    import concourse.tile as tile
    from concourse import bacc, mybir

    f16 = mybir.dt.float16
    f32 = mybir.dt.float32
    i32 = mybir.dt.int32

    nc = bacc.Bacc("TRN2", target_bir_lowering=False, debug=debug)

    xg_d = nc.dram_tensor("xg", [128, int(GOFF[-1])], f16, kind="ExternalInput")
    wl_d = nc.dram_tensor("wl", [128, NCHUNKS, O], f16, kind="ExternalInput")
    out_d = nc.dram_tensor("outt", [16, 128, 4 * O], f16, kind="ExternalOutput")

    with tile.TileContext(nc) as tc:
        with (
            tc.tile_pool(name="wpool", bufs=1) as wpool,
            tc.tile_pool(name="lay", bufs=3) as laypool,
            tc.tile_pool(name="xep", bufs=6) as xep,
            tc.tile_pool(name="zp", bufs=30) as zp,
            tc.tile_pool(name="outp", bufs=4) as outp,
            tc.tile_pool(name="ps", bufs=8, space=bass.MemorySpace.PSUM) as psp,
        ):
            wl_sb = wpool.tile([128, NCHUNKS, O], f16)
            nc.sync.dma_start(wl_sb[:], wl_d[:])

            ot_idx = 0
            blk_seq = 0
            prev_lay = None
            for g, wg in enumerate(GROUPS):
                nlay = GNLAY[g]
                lay = laypool.tile([128, NLAY * WMAX], f16)

                def lsl(li, w=wg):
                    return lay[:, li * w : (li + 1) * w]

                o0 = int(GOFF[g])
                if g == 0:
                    # minimal first sub-DMA (XA0+YA0) so chunk 0's mul and the
                    # first PE chain start as early as possible
                    nc.sync.dma_start(lay[:, 0 : 2 * wg], xg_d[:, o0 : o0 + 2 * wg])
                    nc.sync.dma_start(
                        lay[:, 2 * wg : 6 * wg], xg_d[:, o0 + 2 * wg : o0 + 6 * wg]
                    )
                    nc.sync.dma_start(
                        lay[:, 6 * wg : nlay * wg], xg_d[:, o0 + 6 * wg : o0 + nlay * wg]
                    )
                elif wg >= 1024:
                    # split so early (even-c) chunks can start sooner
                    nc.sync.dma_start(lay[:, 0 : 6 * wg], xg_d[:, o0 : o0 + 6 * wg])
                    nc.sync.dma_start(
                        lay[:, 6 * wg : nlay * wg], xg_d[:, o0 + 6 * wg : o0 + nlay * wg]
                    )
                else:
                    nc.sync.dma_start(lay[:, 0 : nlay * wg], xg_d[:, o0 : o0 + nlay * wg])

                xe = {}
                z = {}

                def mul(c):
                    zc = zp.tile([128, WMAX], f16, tag="z")
                    xi, yi = CHUNK_LAYS[c]
                    if c >= 10 and nlay == 9:
                        xin = xe[c - 10][:, 0:wg]
                    else:
                        xin = lsl(xi)
                    nc.vector.tensor_mul(zc[:, 0:wg], lsl(yi), xin)
                    z[c] = zc

                for c in MORDER[:5]:
                    mul(c)
                if nlay == 9:
                    for cb in range(3):
                        t = xep.tile([128, WMAX], f16, tag="xe")
                        nc.vector.stream_shuffle(
                            t[:, 0:wg].bitcast(i32),
                            lsl(LAY_XIL).bitcast(i32),
                            _shuffle_mask(cb),
                        )
                        xe[cb] = t
                for c in MORDER[5:]:
                    mul(c)

                for k4 in range(wg // 512):
                    ot = outp.tile([128, 4 * O], f16)
                    for kk in range(4):
                        blk = 4 * k4 + kk
                        ps = psp.tile([128, 512], f32)
                        r0 = blk_seq % NCHUNKS
                        blk_seq += 1
                        rot = MORDER[r0:] + MORDER[:r0]
                        sl = slice(blk * BLK, (blk + 1) * BLK)
                        for i, c in enumerate(rot):
                            nc.tensor.matmul(
                                ps[:, 0:O], z[c][:, sl], wl_sb[:, c, :],
                                start=(i == 0), stop=(i == NCHUNKS - 1),
                            )
                        nc.scalar.copy(ot[:, kk * O : (kk + 1) * O], ps[:, 0:O])
                    nc.scalar.dma_start(out_d[ot_idx], ot[:])
                    ot_idx += 1

    nc.compile()
    return nc


def prepare_in_maps(x: np.ndarray, y: np.ndarray, W: np.ndarray):
    wl = _stage_w(W)
    x_cores = x.reshape(NCORES, S_PER_CORE, N)
    y_cores = y.reshape(NCORES, S_PER_CORE, M)
    in_maps = []
    for i in range(NCORES):
        xg = _stage_core_inputs(x_cores[i], y_cores[i])
        in_maps.append({"xg": xg, "wl": wl})
    return in_maps


def kernel(x: np.ndarray, y: np.ndarray, W: np.ndarray) -> np.ndarray:
    from concourse.bass_utils import run_bass_kernel_spmd

    assert x.shape == (BS, DIM, N) and y.shape == (BS, DIM, M)
    assert W.shape == (O, N * M)

    in_maps = prepare_in_maps(x, y, W)
    nc = build_nc()
    res = run_bass_kernel_spmd(nc, in_maps, core_ids=list(range(NCORES)))

    outs = [unpack_out(res.results[i]["outt"]) for i in range(NCORES)]
    return np.concatenate(outs, axis=0).reshape(BS, DIM, O)


if __name__ == "__main__":
    xs = np.random.randn(BS, DIM, N).astype(np.float32)
    ys = np.random.randn(BS, DIM, M).astype(np.float32)
    Ws = (np.random.randn(O, N * M) * (1.0 / np.sqrt(N * M))).astype(np.float32)
    out = kernel(xs, ys, Ws)
    print(out.shape, out.dtype)


# revision 14
# speedup vs baseline: 1.0404x; 1.0171x over previous
"""Trainium2 Bass kernel for nn_CINLayer: out[b,d,o] = sum_{n,m} x[b,d,n]*y[b,d,m]*W[o,n*M+m].

Strategy (8-core data parallel over batch):
  Per sample s, out[s,o] = sum_k Z[k,s] * Wl[k,o] with Z[k,s] = x[s,n(k)]*y[s,m(k)].
  The 1600 (n,m) products are covered by 13 chunks of 128 rows, each chunk a
  product set P x Q so its Z is ONE DVE elementwise multiply of two broadcast
  layouts (no per-chunk shuffling in steady state):
    part A (c=2a+b<10):  row r: (n,m) = (8a + r//16, 16b + r%16)
        z_c = XA_a * YA_b,  XA_a[r]=xT[8a+r//16], YA_b[r]=yT[16b+r%16]
    part B (c=10+cb):    row r=32j+8a'+m'': (n,m) = (16cb+4a'+j, 32+m'')
        [n>=40 rows zeroed in W]
        z_c = XB_cb * YB,  XB_cb[r]=xT[16cb+4(r%32//8)+r//32], YB[r]=yT[32+r%8]
  All 11 layouts are host-staged and DMA'd per sample group as one contiguous
  DRAM block (KB-scale per-partition rows -> near-peak HBM BW). The first two
  (small) groups instead stage 9 layouts and build XB_cb by int32-bitcast DVE
  stream_shuffles of an interleaved base XIL[32j+i]=xT[4i+j] -- during fill
  HBM bandwidth, not DVE, is the binding constraint. Group widths are small at
  the start so the first z chunks (and PE) start as early as possible; the
  first sub-DMA carries only XA0+YA0 for the same reason.
  PE runs z-stationary matmuls: lhsT = z chunk slice [128k, 128 samples]
  (128-wide f16 stationary -> fast weight load, hidden under the previous
  matmul), moving = W chunk [128k, 200o], accumulating psum[128 samples, 200o]
  over the 13 chunks. This avoids the o=200 -> 128+72 two-pass padding of the
  W-stationary form (89us -> 78us of PE time; measured 94ns/matmul ~= the
  LDW+MM production roofline). PSUM tiles are bank-padded to [128,512]f32.
  Output: psum -> f16 SBUF tiles packing 4 sample-blocks [128, 800] -> HBM.
  Measured: 103.2us HW exec (vs 183.7us baseline), rel err 5.7e-4.
"""

import numpy as np

BS, DIM, N, M, O = 2048, 32, 40, 40, 200
NCORES = 8
S_PER_CORE = BS * DIM // NCORES  # 8192
NCHUNKS = 13
NLAY = 11                  # staged: XA0,YA0,XA1..XA4,YA1,YB,XB0..XB2
LAY_YB = 7
LAY_XIL = 8                # in 9-layout (shuffle) mode, slot 8 holds XIL
GROUPS = [512, 512] + [1024] * 7
# early groups use the 9-layout shuffle mode (fewer input bytes while the
# pipeline fills and HBM bandwidth is the binding constraint)
GNLAY = [9 if g < 2 else NLAY for g in range(len(GROUPS))]
assert sum(GROUPS) == S_PER_CORE
GOFF = np.concatenate([[0], np.cumsum([nl * w for nl, w in zip(GNLAY, GROUPS)])])
BLK = 128
WMAX = 1024
F16 = np.float16

# chunk c -> (x layout index, y layout index)
_XA_IDX = [0, 2, 3, 4, 5]
CHUNK_LAYS = [(_XA_IDX[c // 2], 1 if c % 2 == 0 else 6) for c in range(10)] + [
    (8 + cb, LAY_YB) for cb in range(3)
]
# mul issue order: even-c (sub-DMA 1), odd-c, then part B
MORDER = [0, 2, 4, 6, 8, 1, 3, 5, 7, 9, 10, 11, 12]
GPSIMD_MULS = frozenset()


def _shuffle_mask(cb: int):
    return [4 * cb + (rp // 8) for rp in range(32)]


def _chunk_row_to_nm(c: int, r: int):
    """Chunk c (0..12), row r (0..127) -> (n, m) or None (zero pad)."""
    if c < 10:
        return 8 * (c // 2) + r // 16, 16 * (c % 2) + r % 16
    cb = c - 10
    j, rp = divmod(r, 32)
    ap, mpp = divmod(rp, 8)
    n = 16 * cb + 4 * ap + j
    if n >= N:
        return None
    return n, 32 + mpp


def _stage_w(W: np.ndarray) -> np.ndarray:
    """W [O, N*M] f32 -> wl [128, NCHUNKS, O] f16 (z-stationary moving operand)."""
    Wr = W.reshape(O, N, M)
    wl = np.zeros((128, NCHUNKS, O), dtype=F16)
    for c in range(NCHUNKS):
        for r in range(128):
            nm = _chunk_row_to_nm(c, r)
            if nm is not None:
                wl[r, c, :] = Wr[:, nm[0], nm[1]].astype(F16)
    return wl


def _lay_row_maps():
    """11 staged layouts: list of (which, idx[128]) with idx=-1 meaning zero row."""
    r = np.arange(128)
    xa = [("x", 8 * a + r // 16) for a in range(5)]
    ya = [("y", 16 * b + r % 16) for b in range(2)]
    yb = ("y", 32 + r % 8)
    j, rp = r // 32, r % 32
    ap = rp // 8
    xb = []
    for cb in range(3):
        idx = 16 * cb + 4 * ap + j
        xb.append(("x", np.where(idx < N, idx, -1)))
    xil = ("x", np.where(rp < 10, 4 * rp + j, -1))
    return [xa[0], ya[0], xa[1], xa[2], xa[3], xa[4], ya[1], yb] + xb + [xil]


_LAY_MAPS = _lay_row_maps()


def _stage_core_inputs(x_flat: np.ndarray, y_flat: np.ndarray) -> np.ndarray:
    """x_flat, y_flat [S_PER_CORE, 40] f32 -> xg [128, sum(NLAY*Wg)] f16."""
    xT = np.ascontiguousarray(x_flat.T).astype(F16)  # [40, S]
    yT = np.ascontiguousarray(y_flat.T).astype(F16)  # [40, S]
    src = {"x": xT, "y": yT}
    lays = np.empty((len(_LAY_MAPS), 128, S_PER_CORE), dtype=F16)
    for li, (which, idx) in enumerate(_LAY_MAPS):
        t = src[which]
        lays[li] = np.where((idx >= 0)[:, None], t[np.clip(idx, 0, N - 1)], F16(0))
    xg = np.empty((128, GOFF[-1]), dtype=F16)
    s0 = 0
    for g, w in enumerate(GROUPS):
        sel = list(range(8)) + ([11] if GNLAY[g] == 9 else [8, 9, 10])
        blk = lays[sel][:, :, s0 : s0 + w].transpose(1, 0, 2)
        xg[:, GOFF[g] : GOFF[g + 1]] = blk.reshape(128, GNLAY[g] * w)
        s0 += w
    return xg


def unpack_out(outt: np.ndarray) -> np.ndarray:
    """outt [16, 128, 4*O] f16 -> [S_PER_CORE, O] f32."""
    o4 = outt.reshape(-1, 128, 4, O).transpose(0, 2, 1, 3)
    return o4.reshape(S_PER_CORE, O).astype(np.float32)


def build_nc(debug: bool = False):
    """Build the per-core Bass/Tile module."""
    import concourse.bass as bass
    import concourse.tile as tile
    from concourse import bacc, mybir

    f16 = mybir.dt.float16
    f32 = mybir.dt.float32
    i32 = mybir.dt.int32

    nc = bacc.Bacc("TRN2", target_bir_lowering=False, debug=debug)

    xg_d = nc.dram_tensor("xg", [128, int(GOFF[-1])], f16, kind="ExternalInput")
    wl_d = nc.dram_tensor("wl", [128, NCHUNKS, O], f16, kind="ExternalInput")
    out_d = nc.dram_tensor("outt", [16, 128, 4 * O], f16, kind="ExternalOutput")

    with tile.TileContext(nc) as tc:
        with (
            tc.tile_pool(name="wpool", bufs=1) as wpool,
            tc.tile_pool(name="lay", bufs=3) as laypool,
            tc.tile_pool(name="xep", bufs=6) as xep,
            tc.tile_pool(name="zp", bufs=30) as zp,
            tc.tile_pool(name="outp", bufs=4) as outp,
            tc.tile_pool(name="ps", bufs=8, space=bass.MemorySpace.PSUM) as psp,
        ):
            wl_sb = wpool.tile([128, NCHUNKS, O], f16)
            nc.sync.dma_start(wl_sb[:], wl_d[:])

            ot_idx = 0
            blk_seq = 0
            prev_lay = None
            for g, wg in enumerate(GROUPS):
                nlay = GNLAY[g]
                lay = laypool.tile([128, NLAY * WMAX], f16)

                def lsl(li, w=wg):
                    return lay[:, li * w : (li + 1) * w]

                o0 = int(GOFF[g])
                if g == 0:
                    # minimal first sub-DMA (XA0+YA0) so chunk 0's mul and the
                    # first PE chain start as early as possible
                    nc.sync.dma_start(lay[:, 0 : 2 * wg], xg_d[:, o0 : o0 + 2 * wg])
                    nc.sync.dma_start(
                        lay[:, 2 * wg : 6 * wg], xg_d[:, o0 + 2 * wg : o0 + 6 * wg]
                    )
                    nc.sync.dma_start(
                        lay[:, 6 * wg : nlay * wg], xg_d[:, o0 + 6 * wg : o0 + nlay * wg]
                    )
                elif wg >= 1024:
                    # split so early (even-c) chunks can start sooner
                    nc.sync.dma_start(lay[:, 0 : 6 * wg], xg_d[:, o0 : o0 + 6 * wg])
                    nc.sync.dma_start(
                        lay[:, 6 * wg : nlay * wg], xg_d[:, o0 + 6 * wg : o0 + nlay * wg]
                    )
                else:
                    nc.sync.dma_start(lay[:, 0 : nlay * wg], xg_d[:, o0 : o0 + nlay * wg])

                xe = {}
                z = {}

                def mul(c):
                    zc = zp.tile([128, WMAX], f16, tag="z")
                    xi, yi = CHUNK_LAYS[c]
                    if c >= 10 and nlay == 9:
                        xin = xe[c - 10][:, 0:wg]
                    else:
                        xin = lsl(xi)
                    nc.vector.tensor_mul(zc[:, 0:wg], lsl(yi), xin)
                    z[c] = zc

                for c in MORDER[:5]:
                    mul(c)
                if nlay == 9:
                    for cb in range(3):
                        t = xep.tile([128, WMAX], f16, tag="xe")
                        nc.vector.stream_shuffle(
                            t[:, 0:wg].bitcast(i32),
                            lsl(LAY_XIL).bitcast(i32),
                            _shuffle_mask(cb),
                        )
                        xe[cb] = t
                for c in MORDER[5:]:
                    mul(c)

                for k4 in range(wg // 512):
                    ot = outp.tile([128, 4 * O], f16)
                    for kk in range(4):
                        blk = 4 * k4 + kk
                        ps = psp.tile([128, 512], f32)
                        r0 = blk_seq % NCHUNKS
                        blk_seq += 1
                        rot = MORDER[r0:] + MORDER[:r0]
                        sl = slice(blk * BLK, (blk + 1) * BLK)
                        for i, c in enumerate(rot):
                            nc.tensor.matmul(
                                ps[:, 0:O], z[c][:, sl], wl_sb[:, c, :],
                                start=(i == 0), stop=(i == NCHUNKS - 1),
                            )
                        nc.scalar.copy(ot[:, kk * O : (kk + 1) * O], ps[:, 0:O])
                    nc.scalar.dma_start(out_d[ot_idx], ot[:])
                    ot_idx += 1

    nc.compile()
    return nc


def prepare_in_maps(x: np.ndarray, y: np.ndarray, W: np.ndarray):
    wl = _stage_w(W)
    x_cores = x.reshape(NCORES, S_PER_CORE, N)
    y_cores = y.reshape(NCORES, S_PER_CORE, M)
    in_maps = []
    for i in range(NCORES):
        xg = _stage_core_inputs(x_cores[i], y_cores[i])
        in_maps.append({"xg": xg, "wl": wl})
    return in_maps


def kernel(x: np.ndarray, y: np.ndarray, W: np.ndarray) -> np.ndarray:
    from concourse.bass_utils import run_bass_kernel_spmd

    assert x.shape == (BS, DIM, N) and y.shape == (BS, DIM, M)
    assert W.shape == (O, N * M)

    in_maps = prepare_in_maps(x, y, W)
    nc = build_nc()
    res = run_bass_kernel_spmd(nc, in_maps, core_ids=list(range(NCORES)))

    outs = [unpack_out(res.results[i]["outt"]) for i in range(NCORES)]
    return np.concatenate(outs, axis=0).reshape(BS, DIM, O)


if __name__ == "__main__":
    xs = np.random.randn(BS, DIM, N).astype(np.float32)
    ys = np.random.randn(BS, DIM, M).astype(np.float32)
    Ws = (np.random.randn(O, N * M) * (1.0 / np.sqrt(N * M))).astype(np.float32)
    out = kernel(xs, ys, Ws)
    print(out.shape, out.dtype)
